# revision 9
# baseline (speedup 1.0000x reference)
"""BitMoE (dense 8-expert BitNet-style MoE) on 8 Trainium2 NeuronCores.

Expert-parallel: core c computes expert c for all 4096 tokens; a 4-chunk
ReduceScatter sums the gate-weighted expert contributions across cores and
each core returns its token shards.

Math notes (why this matches the fp32 reference within fp32 noise):
  - act_quant(rmsnorm(v)) == round(v * 127/absmax(v)) * (absmax(v)*sqrt(N) /
    (||v||*127)) whenever the rms-normed absmax >= 1e-5 (always true for
    nonzero v).  The int8 grid needs only absmax(v); the norm enters only
    through the dequant scale, folded into downstream fp32 scale factors.
  - int8-valued activations and {-1,0,1} ternary weights are both exactly
    representable in bf16, so the bf16 matmuls + fp32 PSUM accumulation are
    exact; all rounding error lives in fp32 scale factors.
  - round-to-nearest-even via the 1.5*2^23 magic constant fused into the ACT
    engine's scale*x+bias (single fp32 rounding).
"""

import contextlib
import numpy as np

import concourse.bass as bass
import concourse.bacc as bacc
import concourse.tile as tile
from concourse import mybir
from concourse import bass_utils
from concourse import bass_isa

F32 = mybir.dt.float32
BF16 = mybir.dt.bfloat16
AX = mybir.AxisListType
ALU = mybir.AluOpType
ACT_F = mybir.ActivationFunctionType

DIM = 1024
HID = 4096
E = 8
NTOK = 4096  # B*S = 2*2048
P = 128
NT = NTOK // P       # 32 token tiles
KD = DIM // P        # 8 contraction chunks over DIM
KH = HID // P        # 32 contraction chunks over HID
NRS = 4              # ReduceScatter chunks
MAGIC = 12582912.0   # 1.5*2^23: adding to |v|<2^22 rounds v to int (RNE)
SQD = float(np.sqrt(DIM))
SQH = float(np.sqrt(HID))

_prog_cache = {}


def _emit(tc, io, with_bias):
    nc = tc.nc
    st = contextlib.ExitStack()
    NTOK_L = io["x"].shape[0]
    NT = NTOK_L // P

    x_t = io["x"].rearrange("(t p) d -> t p d", p=P)          # [32,128,1024]
    w1_t = io["w1"].rearrange("(t p) d -> t p d", p=P)        # [32,128,1024]
    w2_t = io["w2"].rearrange("(t p) h -> t p h", p=P)        # [8,128,4096]
    gwp = io["gwp"]                                           # [16,1024]
    opre_t = io["opre"].rearrange("(t p) d -> t p d", p=P)    # [32,128,1024]
    part_t = io["part"].rearrange("(t p) d -> t p d", p=P)    # [32,128,1024]

    stats = st.enter_context(tc.tile_pool(name="stats", bufs=1))
    resid = st.enter_context(tc.tile_pool(name="resid", bufs=1))
    f32w = st.enter_context(tc.tile_pool(name="f32w", bufs=5))
    bf16w = st.enter_context(tc.tile_pool(name="bf16w", bufs=3))
    apool = st.enter_context(tc.tile_pool(name="apool", bufs=1))
    aqpool = st.enter_context(tc.tile_pool(name="aqpool", bufs=1))
    aqtp = st.enter_context(tc.tile_pool(name="aqtp", bufs=2))
    xqtp = st.enter_context(tc.tile_pool(name="xqtp", bufs=2))
    ps1 = st.enter_context(tc.tile_pool(name="ps1", bufs=2, space="PSUM"))
    ps2 = st.enter_context(tc.tile_pool(name="ps2", bufs=1, space="PSUM"))
    psl = st.enter_context(tc.tile_pool(name="psl", bufs=2, space="PSUM"))

    def f32t():
        return f32w.tile([P, DIM], F32, tag="f32t", name="f32t")

    def bf16t():
        return bf16w.tile([P, DIM], BF16, tag="bf16t", name="bf16t")

    # ---------------- resident tensors ----------------
    t1T = resid.tile([P, KD, HID], BF16, tag="t1T")    # w1q^T: [d%128, d//128, h]
    t2T = resid.tile([P, KH, DIM], BF16, tag="t2T")    # w2q^T: [h%128, h//128, d]
    gwqT = resid.tile([P, KD, 16], BF16, tag="gwqT")   # gw_pad^T

    # ---------------- per-token stats ----------------
    amx = stats.tile([P, NT], F32, tag="amx")     # absmax(x) per token
    sqx = stats.tile([P, NT], F32, tag="sqx")     # sum(x^2)
    qsx = stats.tile([P, NT], F32, tag="qsx")     # 127/absmax(x)
    s1c = stats.tile([P, NT], F32, tag="s1c")     # gelu input scale = sx*m1
    sgc = stats.tile([P, NT], F32, tag="sgc")     # gate logit scale = sx*mg
    ama = stats.tile([P, NT], F32, tag="ama")     # absmax(gelu) per token
    sqa = stats.tile([P, NT], F32, tag="sqa")     # sum(gelu^2)
    comb = stats.tile([P, NT], F32, tag="comb")   # g * sqrt(H)/||a||
    logits = stats.tile([P, NT, E], F32, tag="logits")
    wabs = stats.tile([P, KH + 32], F32, tag="wabs")  # |w| col-partials
    m1c = stats.tile([P, 1], F32, tag="m1c")      # clip(mean|w1|, 1e-5)
    m2c = stats.tile([P, 1], F32, tag="m2c")
    mgc = stats.tile([P, 1], F32, tag="mgc")
    s1r = stats.tile([P, 1], F32, tag="s1r")      # 1/clip(mean|w1|)
    s2r = stats.tile([P, 1], F32, tag="s2r")
    tmpc = stats.tile([P, NT], F32, tag="tmpc")
    tmpc2 = stats.tile([P, NT], F32, tag="tmpc2")

    if with_bias:
        b1b = resid.tile([P, HID], F32, tag="b1b")
        b2b = resid.tile([P, DIM], F32, tag="b2b")
        gbb = resid.tile([P, E], F32, tag="gbb")
        for name, t in (("b1", b1b), ("b2", b2b), ("gb", gbb)):
            src = io[name]
            bc = bass.AP(tensor=src.tensor, offset=src.offset,
                         ap=[[0, P]] + [list(a) for a in src.ap])
            nc.gpsimd.dma_start(out=t[:], in_=bc)

    # ---------------- gate weight prep (tiny) ----------------
    gwt = stats.tile([16, DIM], F32, tag="gwt")
    nc.sync.dma_start(out=gwt[:], in_=gwp[:])
    gwa = stats.tile([16, 1], F32, tag="gwa")
    nc.vector.tensor_reduce(gwa[:], gwt[:], axis=AX.X, op=ALU.add,
                            apply_absolute_value=True)
    gwa2 = stats.tile([16, 1], F32, tag="gwa2")
    nc.gpsimd.partition_all_reduce(gwa2[:], gwa[:], channels=16,
                                   reduce_op=bass_isa.ReduceOp.add)
    nc.vector.tensor_scalar(gwa2[:], gwa2[:], 1.0 / (E * DIM), 1e-5, ALU.mult,
                            ALU.max)
    gwr = stats.tile([16, 1], F32, tag="gwr")
    nc.vector.reciprocal(gwr[:], gwa2[:])
    nc.gpsimd.partition_broadcast(mgc[:], gwa2[0:1, :])
    ug = gwt
    nc.vector.tensor_scalar(ug[:], gwt[:], gwr[:], MAGIC, ALU.mult, ALU.add)
    nc.vector.tensor_scalar(ug[:], ug[:], MAGIC + 1.0, MAGIC - 1.0, ALU.min,
                            ALU.max)
    gq = stats.tile([16, DIM], BF16, tag="gq")
    nc.scalar.activation(gq[:], ug[:], ACT_F.Copy, bias=-MAGIC, scale=1.0)
    nc.sync.dma_start_transpose(gwqT[:], gq[:])

    # ---------------- expert weight abs-mean pass ----------------
    for i in range(KH):
        wt = f32t()
        nc.sync.dma_start(out=wt[:], in_=w1_t[i])
        nc.vector.tensor_reduce(wabs[:, i:i + 1], wt[:], axis=AX.X, op=ALU.add,
                                apply_absolute_value=True)
    for i in range(KD):
        for q in range(4):
            wt = f32t()
            nc.sync.dma_start(out=wt[:], in_=w2_t[i, :, q * DIM:(q + 1) * DIM])
            j = KH + i * 4 + q
            nc.vector.tensor_reduce(wabs[:, j:j + 1], wt[:], axis=AX.X,
                                    op=ALU.add, apply_absolute_value=True)

    w1s = stats.tile([P, 1], F32, tag="w1s")
    w2s = stats.tile([P, 1], F32, tag="w2s")
    nc.vector.tensor_reduce(w1s[:], wabs[:, 0:KH], axis=AX.X, op=ALU.add)
    nc.vector.tensor_reduce(w2s[:], wabs[:, KH:KH + 32], axis=AX.X,
                            op=ALU.add)
    nc.gpsimd.partition_all_reduce(m1c[:], w1s[:], channels=P,
                                   reduce_op=bass_isa.ReduceOp.add)
    nc.gpsimd.partition_all_reduce(m2c[:], w2s[:], channels=P,
                                   reduce_op=bass_isa.ReduceOp.add)
    nc.vector.tensor_scalar(m1c[:], m1c[:], 1.0 / (HID * DIM), 1e-5, ALU.mult,
                            ALU.max)
    nc.vector.tensor_scalar(m2c[:], m2c[:], 1.0 / (HID * DIM), 1e-5, ALU.mult,
                            ALU.max)
    nc.vector.reciprocal(s1r[:], m1c[:])
    nc.vector.reciprocal(s2r[:], m2c[:])

    # ---------------- x stats pass ----------------
    for t in range(NT):
        xt = f32t()
        nc.sync.dma_start(out=xt[:], in_=x_t[t])
        nc.vector.tensor_reduce(amx[:, t:t + 1], xt[:], axis=AX.X, op=ALU.max,
                                apply_absolute_value=True)
        scr = f32t()
        nc.scalar.activation(scr[:], xt[:], ACT_F.Square,
                             accum_out=sqx[:, t:t + 1])
    # qsx = 127/max(absmax, tiny)
    nc.vector.tensor_scalar(qsx[:], amx[:], 1e-30, None, ALU.max)
    nc.vector.reciprocal(qsx[:], qsx[:])
    nc.vector.tensor_scalar(qsx[:], qsx[:], 127.0, None, ALU.mult)
    # sx = absmax*sqrt(D)/(||x||*127); s1c = sx*m1; sgc = sx*mg
    nc.vector.tensor_scalar(tmpc[:], sqx[:], 1e-24, None, ALU.max)
    nc.scalar.activation(tmpc2[:], tmpc[:], ACT_F.Sqrt)
    nc.vector.reciprocal(tmpc2[:], tmpc2[:])          # 1/||x||
    nc.vector.tensor_tensor(tmpc[:], amx[:], tmpc2[:], op=ALU.mult)
    nc.vector.tensor_scalar(s1c[:], tmpc[:], m1c[:], SQD / 127.0, ALU.mult,
                            ALU.mult)
    nc.vector.tensor_scalar(sgc[:], tmpc[:], mgc[:], SQD / 127.0, ALU.mult,
                            ALU.mult)

    # ---------------- ternarize + transpose weights ----------------
    def ternarize(dst_view, src_ap, sAP):
        wt = f32t()
        nc.sync.dma_start(out=wt[:], in_=src_ap)
        u = f32t()
        nc.vector.tensor_scalar(u[:], wt[:], sAP, MAGIC, ALU.mult, ALU.add)
        nc.vector.tensor_scalar(u[:], u[:], MAGIC + 1.0, MAGIC - 1.0, ALU.min,
                                ALU.max)
        tt = bf16t()
        nc.scalar.activation(tt[:], u[:], ACT_F.Copy, bias=-MAGIC, scale=1.0)
        nc.sync.dma_start_transpose(dst_view, tt[:])

    for i in range(KH):
        ternarize(t1T[:, :, i * P:(i + 1) * P], w1_t[i], s1r[:])
    for i in range(KD):
        for q in range(4):
            ternarize(t2T[:, q * 8:(q + 1) * 8, i * P:(i + 1) * P],
                      w2_t[i, :, q * DIM:(q + 1) * DIM], s2r[:])

    # ---------------- main loop over token tiles ----------------
    HC = 1024          # H columns per psum chunk
    NHC = HID // HC    # 4 chunks
    for t in range(NT):
        # re-load x, quantize to int8-grid bf16, transpose
        xt = f32t()
        nc.sync.dma_start(out=xt[:], in_=x_t[t])
        zq = f32t()
        nc.scalar.activation(zq[:], xt[:], ACT_F.Copy, bias=MAGIC,
                             scale=qsx[:, t:t + 1])
        xq = bf16t()
        nc.vector.tensor_scalar(xq[:], zq[:], -MAGIC, None, ALU.add)
        xqT = xqtp.tile([P, KD, P], BF16, tag="xqT")
        nc.sync.dma_start_transpose(xqT[:], xq[:])

        a = apool.tile([P, HID], F32, tag="a")
        for c in range(NHC):
            ph = ps1.tile([P, HC], F32, tag="ph")
            for k in range(KD):
                for h2 in range(HC // 512):
                    nc.tensor.matmul(
                        ph[:, h2 * 512:(h2 + 1) * 512], xqT[:, k, :],
                        t1T[:, k, c * HC + h2 * 512: c * HC + (h2 + 1) * 512],
                        start=(k == 0), stop=(k == KD - 1))
            if with_bias:
                hb = f32t()
                nc.vector.scalar_tensor_tensor(
                    hb[:], ph[:], s1c[:, t:t + 1],
                    b1b[:, c * HC:(c + 1) * HC], ALU.mult, ALU.add)
                nc.scalar.activation(a[:, c * HC:(c + 1) * HC], hb[:],
                                     ACT_F.Gelu)
            else:
                nc.scalar.activation(a[:, c * HC:(c + 1) * HC], ph[:],
                                     ACT_F.Gelu, scale=s1c[:, t:t + 1])
        # gate logits ride the same stationary weights
        pl = psl.tile([P, E], F32, tag="pl")
        for k in range(KD):
            nc.tensor.matmul(pl[:], xqT[:, k, :], gwqT[:, k, 0:E],
                             start=(k == 0), stop=(k == KD - 1))
        nc.vector.tensor_scalar(logits[:, t, :], pl[:], sgc[:, t:t + 1], None,
                                ALU.mult)

        # stats over a, quantize, transpose
        nc.vector.tensor_reduce(ama[:, t:t + 1], a[:], axis=AX.X, op=ALU.max,
                                apply_absolute_value=True)
        qsa = stats.tile([P, 1], F32, tag="qsa")
        nc.vector.tensor_scalar(qsa[:], ama[:, t:t + 1], 1e-30, None, ALU.max)
        nc.vector.reciprocal(qsa[:], qsa[:])
        nc.vector.tensor_scalar(qsa[:], qsa[:], 127.0, None, ALU.mult)
        aq = aqpool.tile([P, HID], BF16, tag="aq")
        for c in range(NHC):
            zqa = f32t()
            nc.scalar.activation(zqa[:], a[:, c * HC:(c + 1) * HC], ACT_F.Copy,
                                 bias=MAGIC, scale=qsa[:])
            nc.vector.tensor_scalar(aq[:, c * HC:(c + 1) * HC], zqa[:], -MAGIC,
                                    None, ALU.add)
        nc.scalar.activation(a[:], a[:], ACT_F.Square,
                             accum_out=sqa[:, t:t + 1])
        aqT = aqtp.tile([P, KH, P], BF16, tag="aqT")
        nc.sync.dma_start_transpose(aqT[:], aq[:])

        # second matmul: out_pre[tok, D]
        p2 = ps2.tile([P, DIM], F32, tag="p2")
        for k in range(KH):
            for h2 in range(DIM // 512):
                nc.tensor.matmul(p2[:, h2 * 512:(h2 + 1) * 512], aqT[:, k, :],
                                 t2T[:, k, h2 * 512:(h2 + 1) * 512],
                                 start=(k == 0), stop=(k == KH - 1))
        # pre-scale = absmax_a * m2 / 127 (gate & 1/||a|| deferred)
        pts = stats.tile([P, 1], F32, tag="pts")
        nc.vector.tensor_scalar(pts[:], ama[:, t:t + 1], m2c[:], 1.0 / 127.0,
                                ALU.mult, ALU.mult)
        op_t = f32t()
        nc.scalar.activation(op_t[:], p2[:], ACT_F.Copy, scale=pts[:])
        nc.sync.dma_start(out=opre_t[t], in_=op_t[:])

    # ---------------- gates (softmax over experts; ours is column 0) -------
    if with_bias:
        gbt = bass.AP(tensor=gbb.tensor, offset=gbb.offset,
                      ap=[list(gbb.ap[0]), [0, NT], list(gbb.ap[1])])
        nc.vector.tensor_tensor(logits[:], logits[:], gbt, op=ALU.add)
    exps = logits
    nc.scalar.activation(exps[:], logits[:], ACT_F.Exp)
    gsum = stats.tile([P, NT], F32, tag="gsum")
    nc.vector.tensor_reduce(gsum[:], exps[:], axis=AX.X, op=ALU.add)
    nc.vector.reciprocal(gsum[:], gsum[:])
    nc.vector.tensor_tensor(comb[:], exps[:, :, 0], gsum[:], op=ALU.mult)
    # rs = sqrt(H)/||a||;  comb = g * rs
    nc.vector.tensor_scalar(tmpc[:], sqa[:], 1e-24, None, ALU.max)
    nc.scalar.activation(tmpc2[:], tmpc[:], ACT_F.Sqrt)
    nc.vector.reciprocal(tmpc2[:], tmpc2[:])
    nc.vector.tensor_tensor(comb[:], comb[:], tmpc2[:], op=ALU.mult)
    nc.vector.tensor_scalar(comb[:], comb[:], SQH, None, ALU.mult)

    # ---------------- final scale pass ----------------
    for t in range(NT):
        ft = f32t()
        nc.sync.dma_start(out=ft[:], in_=opre_t[t])
        if with_bias:
            nc.vector.scalar_tensor_tensor(ft[:], ft[:], comb[:, t:t + 1],
                                           b2b[:], ALU.mult, ALU.add)
        else:
            nc.vector.tensor_scalar(ft[:], ft[:], comb[:, t:t + 1], None,
                                    ALU.mult)
        nc.sync.dma_start(out=part_t[t], in_=ft[:])

    # ---------------- reduce-scatter + output ----------------
    part_fl = io["part"].rearrange("a b -> (a b)")
    rs_fl = io["rs"].rearrange("a b c -> (a b c)")
    csz = NTOK_L * DIM // NRS
    ssz = csz // E
    rows = NTOK_L // NRS // E
    for ch in range(NRS):
        nc.gpsimd.collective_compute(
            "ReduceScatter", ALU.add,
            replica_groups=[list(range(E))],
            ins=[part_fl[ch * csz:(ch + 1) * csz]],
            outs=[rs_fl[ch * ssz:(ch + 1) * ssz]],
        )
    for ch in range(NRS):
        ot = f32t()
        nc.sync.dma_start(out=ot[:rows, :], in_=io["rs"][ch])
        nc.sync.dma_start(out=io["out"][ch], in_=ot[:rows, :])
    st.close()


def build(with_bias=False, ntok=NTOK):
    key = (bool(with_bias), ntok)
    if key in _prog_cache:
        return _prog_cache[key]
    nc = bacc.Bacc("TRN2", target_bir_lowering=False, debug=False,
                   enable_asserts=False, num_devices=E)
    io = {}
    io["x"] = nc.dram_tensor("x", [ntok, DIM], F32, kind="ExternalInput").ap()
    io["w1"] = nc.dram_tensor("w1", [HID, DIM], F32, kind="ExternalInput").ap()
    io["w2"] = nc.dram_tensor("w2", [DIM, HID], F32, kind="ExternalInput").ap()
    io["gwp"] = nc.dram_tensor("gwp", [16, DIM], F32,
                               kind="ExternalInput").ap()
    if with_bias:
        io["gb"] = nc.dram_tensor("gb", [E], F32, kind="ExternalInput").ap()
        io["b1"] = nc.dram_tensor("b1", [HID], F32, kind="ExternalInput").ap()
        io["b2"] = nc.dram_tensor("b2", [DIM], F32, kind="ExternalInput").ap()
    io["opre"] = nc.dram_tensor("opre", [ntok, DIM], F32, kind="Internal").ap()
    io["part"] = nc.dram_tensor("part", [ntok, DIM], F32, kind="Internal").ap()
    rows = ntok // NRS // E
    io["rs"] = nc.dram_tensor("rs", [NRS, rows, DIM], F32, kind="Internal").ap()
    io["out"] = nc.dram_tensor("out", [NRS, rows, DIM], F32,
                               kind="ExternalOutput").ap()
    with tile.TileContext(nc) as tc:
        _emit(tc, io, with_bias)
    nc.compile()
    _prog_cache[key] = (nc, io)
    return nc, io


def kernel(x, gw, gb, w1, b1, w2, b2, _trace=False):
    x = np.ascontiguousarray(np.asarray(x, dtype=np.float32).reshape(NTOK,
                                                                     DIM))
    gw = np.asarray(gw, np.float32)
    gb = np.asarray(gb, np.float32)
    w1 = np.asarray(w1, np.float32)
    b1 = np.asarray(b1, np.float32)
    w2 = np.asarray(w2, np.float32)
    b2 = np.asarray(b2, np.float32)
    with_bias = bool(gb.any() or b1.any() or b2.any())
    nc, io = build(with_bias)
    in_maps = []
    for c in range(E):
        gwr = np.roll(gw, -c, axis=0)  # this core's expert at row 0
        gwp = np.zeros((16, DIM), np.float32)
        gwp[:E] = gwr
        m = {"x": x, "w1": np.ascontiguousarray(w1[c]),
             "w2": np.ascontiguousarray(w2[c]), "gwp": gwp}
        if with_bias:
            m["gb"] = np.ascontiguousarray(np.roll(gb, -c))
            m["b1"] = np.ascontiguousarray(b1[c])
            m["b2"] = np.ascontiguousarray(b2[c])
        in_maps.append(m)
    res = bass_utils.run_bass_kernel_spmd(nc, in_maps,
                                          core_ids=list(range(E)),
                                          trace=_trace)
    rows = NTOK // NRS // E
    full = np.zeros((NTOK, DIM), np.float32)
    for c in range(E):
        o = res.results[c]["out"]  # [NRS, rows, DIM]
        for j in range(NRS):
            r0 = j * (NTOK // NRS) + c * rows
            full[r0:r0 + rows] = o[j]
    out = full.reshape(2, 2048, DIM)
    if _trace:
        return out, res
    return out


# revision 11
# speedup vs baseline: 1.4471x; 1.4471x over previous
"""BitMoE (dense 8-expert BitNet-style MoE) on 8 Trainium2 NeuronCores.

Expert-parallel: core c computes expert c for all 4096 tokens; a 4-chunk
ReduceScatter sums the gate-weighted expert contributions across cores and
each core returns its token shards.

Math notes (why this matches the fp32 reference within fp32 noise):
  - act_quant(rmsnorm(v)) == round(v * 127/absmax(v)) * (absmax(v)*sqrt(N) /
    (||v||*127)) whenever the rms-normed absmax >= 1e-5 (always true for
    nonzero v).  The int8 grid needs only absmax(v); the norm enters only
    through the dequant scale, folded into downstream fp32 scale factors.
  - int8-valued activations and {-1,0,1} ternary weights are both exactly
    representable in bf16, so the bf16 matmuls + fp32 PSUM accumulation are
    exact; all rounding error lives in fp32 scale factors.
  - round-to-nearest-even via the 1.5*2^23 magic constant fused into the ACT
    engine's scale*x+bias (single fp32 rounding).
  - 1/sqrt via DVE Newton iterations (bit-trick seed + 3 steps, ~1e-7 rel)
    instead of ACT Sqrt, avoiding per-tile activation-table reloads.

Performance structure:
  - mm2 of tile t-1 is emitted after mm1 of tile t: the PE sequencer is
    in-order, so mm2's wait on the quantize chain hides behind mm1.
  - gate logits ride the x-phase; one batched Exp; all per-token scales are
    folded into the mm2 PSUM-drain copy, so tiles stream straight to the
    ReduceScatter (4 chunks, fired every 8 tiles, overlapping compute).
"""

import contextlib
import numpy as np

import concourse.bass as bass
import concourse.bacc as bacc
import concourse.tile as tile
from concourse import mybir
from concourse import bass_utils
from concourse import bass_isa

F32 = mybir.dt.float32
I32 = mybir.dt.int32
BF16 = mybir.dt.bfloat16
AX = mybir.AxisListType
ALU = mybir.AluOpType
ACT_F = mybir.ActivationFunctionType

DIM = 1024
HID = 4096
E = 8
NTOK = 4096  # B*S = 2*2048
P = 128
KD = DIM // P        # 8 contraction chunks over DIM
KH = HID // P        # 32 contraction chunks over HID
NRS = 4              # ReduceScatter chunks
MAGIC = 12582912.0   # 1.5*2^23: adding to |v|<2^22 rounds v to int (RNE)
SQD = float(np.sqrt(DIM))
SQH = float(np.sqrt(HID))

_prog_cache = {}


def _emit(tc, io, with_bias):
    nc = tc.nc
    st = contextlib.ExitStack()
    NTOK_L = io["x"].shape[0]
    NT = NTOK_L // P

    x_t = io["x"].rearrange("(t p) d -> t p d", p=P)          # [NT,128,1024]
    w1_t = io["w1"].rearrange("(t p) d -> t p d", p=P)        # [32,128,1024]
    w2_t = io["w2"].rearrange("(t p) h -> t p h", p=P)        # [8,128,4096]
    gwp = io["gwp"]                                           # [16,1024]
    xqs_t = io["xqs"].rearrange("(t p) d -> t p d", p=P)      # xqT spill
    part_t = io["part"].rearrange("(t p) d -> t p d", p=P)    # [NT,128,1024]

    stats = st.enter_context(tc.tile_pool(name="stats", bufs=1))
    resid = st.enter_context(tc.tile_pool(name="resid", bufs=1))
    xtp = st.enter_context(tc.tile_pool(name="xtp", bufs=2))
    wlp = st.enter_context(tc.tile_pool(name="wlp", bufs=2))
    f32w = st.enter_context(tc.tile_pool(name="f32w", bufs=2))
    bf16w = st.enter_context(tc.tile_pool(name="bf16w", bufs=3))
    apool = st.enter_context(tc.tile_pool(name="apool", bufs=1))
    aqpool = st.enter_context(tc.tile_pool(name="aqpool", bufs=1))
    aqtp = st.enter_context(tc.tile_pool(name="aqtp", bufs=2))
    xqtp = st.enter_context(tc.tile_pool(name="xqtp", bufs=2))
    ps1 = st.enter_context(tc.tile_pool(name="ps1", bufs=2, space="PSUM"))
    ps2 = st.enter_context(tc.tile_pool(name="ps2", bufs=2, space="PSUM"))

    def xt_tile():
        return xtp.tile([P, DIM], F32, tag="xt", name="xt")

    def wl_tile():
        return wlp.tile([P, DIM], F32, tag="wl", name="wl")

    def f32t():
        return f32w.tile([P, DIM], F32, tag="f32t", name="f32t")

    def bf16t(shape=None):
        return bf16w.tile(shape or [P, DIM], BF16, tag="bf16t", name="bf16t")

    # ---------------- resident tensors ----------------
    t1T = resid.tile([P, KD, HID], BF16, tag="t1T")    # w1q^T [d%128,d//128,h]
    t2T = resid.tile([P, KH, DIM], BF16, tag="t2T")    # w2q^T [h%128,h//128,d]
    gwqT = resid.tile([P, KD, 16], BF16, tag="gwqT")   # gw_pad^T

    # ---------------- per-token stats ----------------
    def col(tag, n=NT):
        return stats.tile([P, n], F32, tag=tag, name=tag)

    amx, sqx, qsx = col("amx"), col("sqx"), col("qsx")
    s1c, sgc = col("s1c"), col("sgc")
    ama, sqa = col("ama"), col("sqa")
    gcol = col("gcol")
    rnx = col("rnx")
    nt1, nt2, nt3 = col("nt1"), col("nt2"), col("nt3")
    logits = stats.tile([P, NT, E], F32, tag="logits", name="logits")
    wabs = stats.tile([P, KH + 32], F32, tag="wabs", name="wabs")
    m1c = col("m1c", 1)
    m2c = col("m2c", 1)
    mgc = col("mgc", 1)
    s1r = col("s1r", 1)
    s2r = col("s2r", 1)
    w1s = col("w1s", 1)
    w2s = col("w2s", 1)
    amac = col("amac", 4)

    if with_bias:
        b1b = resid.tile([P, HID], F32, tag="b1b", name="b1b")
        b2b = resid.tile([P, DIM], F32, tag="b2b", name="b2b")
        gbb = resid.tile([P, E], F32, tag="gbb", name="gbb")
        for name, t in (("b1", b1b), ("b2", b2b), ("gb", gbb)):
            src = io[name]
            bc = bass.AP(tensor=src.tensor, offset=src.offset,
                         ap=[[0, P]] + [list(a) for a in src.ap])
            nc.gpsimd.dma_start(out=t[:], in_=bc)

    def newton_rsqrt(dst, src, n):
        """dst = 1/sqrt(src) elementwise on [P, n] via DVE only."""
        t_i = stats.tile([P, n], I32, tag=f"nw_i{n}", name="nw_i")
        nwa = stats.tile([P, n], F32, tag=f"nwa{n}", name="nwa")
        nc.vector.tensor_scalar(t_i[:], src[:].bitcast(I32), 1, None,
                                ALU.arith_shift_right)
        nc.vector.tensor_scalar(t_i[:], t_i[:], -1, 0x5F3759DF, ALU.mult,
                                ALU.add)
        y = t_i[:].bitcast(F32)
        for _ in range(3):
            nc.vector.tensor_tensor(nwa[:], y, y, op=ALU.mult)
            nc.vector.tensor_tensor(nwa[:], nwa[:], src[:], op=ALU.mult)
            nc.vector.tensor_scalar(nwa[:], nwa[:], -0.5, 1.5,
                                    ALU.mult, ALU.add)
            nc.vector.tensor_tensor(y, y, nwa[:], op=ALU.mult)
        nc.vector.tensor_copy(dst[:], y)

    # ---------------- gate weight prep (tiny) ----------------
    gwt = xtp.tile([16, DIM], F32, tag="xt", name="gwt")
    nc.sync.dma_start(out=gwt[:], in_=gwp[:])
    gwa = stats.tile([16, 1], F32, tag="gwa", name="gwa")
    nc.vector.tensor_reduce(gwa[:], gwt[:], axis=AX.X, op=ALU.add,
                            apply_absolute_value=True)
    gwa2 = stats.tile([16, 1], F32, tag="gwa2", name="gwa2")
    nc.gpsimd.partition_all_reduce(gwa2[:], gwa[:], channels=16,
                                   reduce_op=bass_isa.ReduceOp.add)
    nc.vector.tensor_scalar(gwa2[:], gwa2[:], 1.0 / (E * DIM), 1e-5, ALU.mult,
                            ALU.max)
    gwr = stats.tile([16, 1], F32, tag="gwr", name="gwr")
    nc.vector.reciprocal(gwr[:], gwa2[:])
    nc.gpsimd.partition_broadcast(mgc[:], gwa2[0:1, :])
    ug = xtp.tile([16, DIM], F32, tag="xt", name="ug")
    nc.vector.tensor_scalar(ug[:], gwt[:], gwr[:], MAGIC, ALU.mult, ALU.add)
    nc.vector.tensor_scalar(ug[:], ug[:], MAGIC + 1.0, MAGIC - 1.0, ALU.min,
                            ALU.max)
    gq = bf16t([16, DIM])
    nc.scalar.activation(gq[:], ug[:], ACT_F.Copy, bias=-MAGIC, scale=1.0)
    nc.sync.dma_start_transpose(gwqT[:], gq[:])

    # ---------------- expert weight abs-mean pass ----------------
    for i in range(KH):
        wt = wl_tile()
        nc.sync.dma_start(out=wt[:], in_=w1_t[i])
        nc.vector.tensor_reduce(wabs[:, i:i + 1], wt[:], axis=AX.X, op=ALU.add,
                                apply_absolute_value=True)
    for i in range(KD):
        for q in range(4):
            wt = wl_tile()
            nc.sync.dma_start(out=wt[:], in_=w2_t[i, :, q * DIM:(q + 1) * DIM])
            j = KH + i * 4 + q
            nc.vector.tensor_reduce(wabs[:, j:j + 1], wt[:], axis=AX.X,
                                    op=ALU.add, apply_absolute_value=True)

    nc.vector.tensor_reduce(w1s[:], wabs[:, 0:KH], axis=AX.X, op=ALU.add)
    nc.vector.tensor_reduce(w2s[:], wabs[:, KH:KH + 32], axis=AX.X,
                            op=ALU.add)
    nc.gpsimd.partition_all_reduce(m1c[:], w1s[:], channels=P,
                                   reduce_op=bass_isa.ReduceOp.add)
    nc.gpsimd.partition_all_reduce(m2c[:], w2s[:], channels=P,
                                   reduce_op=bass_isa.ReduceOp.add)
    nc.vector.tensor_scalar(m1c[:], m1c[:], 1.0 / (HID * DIM), 1e-5, ALU.mult,
                            ALU.max)
    nc.vector.tensor_scalar(m2c[:], m2c[:], 1.0 / (HID * DIM), 1e-5, ALU.mult,
                            ALU.max)
    nc.vector.reciprocal(s1r[:], m1c[:])
    nc.vector.reciprocal(s2r[:], m2c[:])

    # ---------------- ternarize + transpose weights ----------------
    def ternarize(dst_view, src_ap, sAP):
        wt = wl_tile()
        nc.sync.dma_start(out=wt[:], in_=src_ap)
        nc.vector.tensor_scalar(wt[:], wt[:], sAP, MAGIC, ALU.mult, ALU.add)
        nc.vector.tensor_scalar(wt[:], wt[:], MAGIC + 1.0, MAGIC - 1.0,
                                ALU.min, ALU.max)
        tt = bf16t()
        nc.scalar.activation(tt[:], wt[:], ACT_F.Copy, bias=-MAGIC, scale=1.0)
        nc.sync.dma_start_transpose(dst_view, tt[:])

    for i in range(KH):
        ternarize(t1T[:, :, i * P:(i + 1) * P], w1_t[i], s1r[:])
    for i in range(KD):
        for q in range(4):
            ternarize(t2T[:, q * 8:(q + 1) * 8, i * P:(i + 1) * P],
                      w2_t[i, :, q * DIM:(q + 1) * DIM], s2r[:])

    # ---------------- x phase: stats, quantize, transpose, gate mm -------
    for t in range(NT):
        xt = xt_tile()
        nc.sync.dma_start(out=xt[:], in_=x_t[t])
        nc.vector.tensor_reduce(amx[:, t:t + 1], xt[:], axis=AX.X, op=ALU.max,
                                apply_absolute_value=True)
        scr = f32t()
        nc.scalar.activation(scr[:], xt[:], ACT_F.Square,
                             accum_out=sqx[:, t:t + 1])
        nc.vector.tensor_scalar(qsx[:, t:t + 1], amx[:, t:t + 1], 1e-30, None,
                                ALU.max)
        nc.vector.reciprocal(qsx[:, t:t + 1], qsx[:, t:t + 1])
        nc.vector.tensor_scalar(qsx[:, t:t + 1], qsx[:, t:t + 1], 127.0, None,
                                ALU.mult)
        zq = f32t()
        nc.scalar.activation(zq[:], xt[:], ACT_F.Copy, bias=MAGIC,
                             scale=qsx[:, t:t + 1])
        xq = bf16t()
        nc.vector.tensor_scalar(xq[:], zq[:], -MAGIC, None, ALU.add)
        xqT = xqtp.tile([P, KD, P], BF16, tag="xqT", name="xqT")
        nc.sync.dma_start_transpose(xqT[:], xq[:])
        nc.sync.dma_start(out=xqs_t[t],
                          in_=xqT[:].rearrange("p a b -> p (a b)"))
        pl = ps1.tile([P, E], F32, tag="ph", name="pl")
        for k in range(KD):
            nc.tensor.matmul(pl[:], xqT[:, k, :], gwqT[:, k, 0:E],
                             start=(k == 0), stop=(k == KD - 1))
        nc.vector.tensor_copy(logits[:, t, :], pl[:])

    # batched scales: rnx = 1/||x||, s1c = sx*m1, sgc = sx*mg
    nc.vector.tensor_scalar(nt1[:], sqx[:], 1e-24, None, ALU.max)
    newton_rsqrt(rnx, nt1, NT)
    nc.vector.tensor_tensor(nt1[:], amx[:], rnx[:], op=ALU.mult)
    nc.vector.tensor_scalar(s1c[:], nt1[:], m1c[:], SQD / 127.0, ALU.mult,
                            ALU.mult)
    nc.vector.tensor_scalar(sgc[:], nt1[:], mgc[:], SQD / 127.0, ALU.mult,
                            ALU.mult)
    # scale logits per tile, add gb, softmax -> our gate column (expert 0)
    for t in range(NT):
        nc.vector.tensor_scalar(logits[:, t, :], logits[:, t, :],
                                sgc[:, t:t + 1], None, ALU.mult)
    if with_bias:
        gbt = bass.AP(tensor=gbb.tensor, offset=gbb.offset,
                      ap=[list(gbb.ap[0]), [0, NT], list(gbb.ap[1])])
        nc.vector.tensor_tensor(logits[:], logits[:], gbt, op=ALU.add)
    nc.scalar.activation(logits[:], logits[:], ACT_F.Exp)
    gsum = col("gsum")
    nc.vector.tensor_reduce(gsum[:], logits[:], axis=AX.X, op=ALU.add)
    nc.vector.reciprocal(gsum[:], gsum[:])
    nc.vector.tensor_tensor(gcol[:], logits[:, :, 0], gsum[:], op=ALU.mult)

    # ---------------- main loop (mm2 skewed one tile behind mm1) ---------
    HC = 1024          # H columns per psum chunk
    NHC = HID // HC    # 4 chunks
    rs_rows = NTOK_L // NRS // E
    tiles_per_rs = NT // NRS
    saved = {}

    def mm1_part(t):
        xqT2 = bf16t([P, KD, P])
        nc.sync.dma_start(out=xqT2[:].rearrange("p a b -> p (a b)"),
                          in_=xqs_t[t])
        a = apool.tile([P, HID], F32, tag="a", name="a")
        for c in range(NHC):
            ph = ps1.tile([P, HC], F32, tag="ph", name="ph")
            for k in range(KD):
                for h2 in range(HC // 512):
                    nc.tensor.matmul(
                        ph[:, h2 * 512:(h2 + 1) * 512], xqT2[:, k, :],
                        t1T[:, k, c * HC + h2 * 512: c * HC + (h2 + 1) * 512],
                        start=(k == 0), stop=(k == KD - 1))
            if with_bias:
                hb = f32t()
                nc.vector.scalar_tensor_tensor(
                    hb[:], ph[:], s1c[:, t:t + 1],
                    b1b[:, c * HC:(c + 1) * HC], ALU.mult, ALU.add)
                nc.scalar.activation(a[:, c * HC:(c + 1) * HC], hb[:],
                                     ACT_F.Gelu)
            else:
                nc.scalar.activation(a[:, c * HC:(c + 1) * HC], ph[:],
                                     ACT_F.Gelu, scale=s1c[:, t:t + 1])
            nc.vector.tensor_reduce(amac[:, c:c + 1],
                                    a[:, c * HC:(c + 1) * HC], axis=AX.X,
                                    op=ALU.max, apply_absolute_value=True)
        nc.vector.tensor_reduce(ama[:, t:t + 1], amac[:], axis=AX.X,
                                op=ALU.max)
        qsa = stats.tile([P, 1], F32, tag="qsa", name="qsa")
        nc.vector.tensor_scalar(qsa[:], ama[:, t:t + 1], 1e-30, None, ALU.max)
        nc.vector.reciprocal(qsa[:], qsa[:])
        nc.vector.tensor_scalar(qsa[:], qsa[:], 127.0, None, ALU.mult)
        aq = aqpool.tile([P, HID], BF16, tag="aq", name="aq")
        for c in range(NHC):
            zqa = f32t()
            nc.scalar.activation(zqa[:], a[:, c * HC:(c + 1) * HC], ACT_F.Copy,
                                 bias=MAGIC, scale=qsa[:])
            nc.vector.tensor_scalar(aq[:, c * HC:(c + 1) * HC], zqa[:], -MAGIC,
                                    None, ALU.add)
        nc.scalar.activation(a[:], a[:], ACT_F.Square,
                             accum_out=sqa[:, t:t + 1])
        aqT = aqtp.tile([P, KH, P], BF16, tag="aqT", name="aqT")
        nc.sync.dma_start_transpose(aqT[:], aq[:])
        saved[t] = aqT

    def mm2_part(t):
        aqT = saved.pop(t)
        p2 = ps2.tile([P, DIM], F32, tag="p2", name="p2")
        for k in range(KH):
            for h2 in range(DIM // 512):
                nc.tensor.matmul(p2[:, h2 * 512:(h2 + 1) * 512], aqT[:, k, :],
                                 t2T[:, k, h2 * 512:(h2 + 1) * 512],
                                 start=(k == 0), stop=(k == KH - 1))
        # full output scale: ama*m2/127 * g * sqrt(H)/||a||
        sq1 = stats.tile([P, 1], F32, tag="sq1", name="sq1")
        nc.vector.tensor_scalar(sq1[:], sqa[:, t:t + 1], 1e-24, None, ALU.max)
        rsn = stats.tile([P, 1], F32, tag="rsn", name="rsn")
        newton_rsqrt(rsn, sq1, 1)
        sc = stats.tile([P, 1], F32, tag="sc", name="sc")
        nc.vector.tensor_scalar(sc[:], ama[:, t:t + 1], m2c[:], SQH / 127.0,
                                ALU.mult, ALU.mult)
        nc.vector.tensor_tensor(sc[:], sc[:], rsn[:], op=ALU.mult)
        nc.vector.tensor_tensor(sc[:], sc[:], gcol[:, t:t + 1], op=ALU.mult)
        ot = f32t()
        nc.scalar.activation(ot[:], p2[:], ACT_F.Copy, scale=sc[:])
        if with_bias:
            nc.vector.scalar_tensor_tensor(ot[:], b2b[:], gcol[:, t:t + 1],
                                           ot[:], ALU.mult, ALU.add)
        nc.sync.dma_start(out=part_t[t], in_=ot[:])
        # fire the ReduceScatter chunk as soon as its tiles are stored
        if (t + 1) % tiles_per_rs == 0:
            ch = t // tiles_per_rs
            part_fl = io["part"].rearrange("a b -> (a b)")
            rs_fl = io["rs"].rearrange("a b c -> (a b c)")
            csz = NTOK_L * DIM // NRS
            ssz = csz // E
            nc.gpsimd.collective_compute(
                "ReduceScatter", ALU.add,
                replica_groups=[list(range(E))],
                ins=[part_fl[ch * csz:(ch + 1) * csz]],
                outs=[rs_fl[ch * ssz:(ch + 1) * ssz]],
            )
            ot2 = f32t()
            nc.sync.dma_start(out=ot2[:rs_rows, :], in_=io["rs"][ch])
            nc.sync.dma_start(out=io["out"][ch], in_=ot2[:rs_rows, :])

    for t in range(NT):
        mm1_part(t)
        if t > 0:
            mm2_part(t - 1)
    mm2_part(NT - 1)
    st.close()


def build(with_bias=False, ntok=NTOK):
    key = (bool(with_bias), ntok)
    if key in _prog_cache:
        return _prog_cache[key]
    nc = bacc.Bacc("TRN2", target_bir_lowering=False, debug=False,
                   enable_asserts=False, num_devices=E)
    io = {}
    io["x"] = nc.dram_tensor("x", [ntok, DIM], F32, kind="ExternalInput").ap()
    io["w1"] = nc.dram_tensor("w1", [HID, DIM], F32, kind="ExternalInput").ap()
    io["w2"] = nc.dram_tensor("w2", [DIM, HID], F32, kind="ExternalInput").ap()
    io["gwp"] = nc.dram_tensor("gwp", [16, DIM], F32,
                               kind="ExternalInput").ap()
    if with_bias:
        io["gb"] = nc.dram_tensor("gb", [E], F32, kind="ExternalInput").ap()
        io["b1"] = nc.dram_tensor("b1", [HID], F32, kind="ExternalInput").ap()
        io["b2"] = nc.dram_tensor("b2", [DIM], F32, kind="ExternalInput").ap()
    io["xqs"] = nc.dram_tensor("xqs", [ntok, DIM], BF16, kind="Internal").ap()
    io["part"] = nc.dram_tensor("part", [ntok, DIM], F32, kind="Internal").ap()
    rows = ntok // NRS // E
    io["rs"] = nc.dram_tensor("rs", [NRS, rows, DIM], F32,
                              kind="Internal").ap()
    io["out"] = nc.dram_tensor("out", [NRS, rows, DIM], F32,
                               kind="ExternalOutput").ap()
    with tile.TileContext(nc) as tc:
        _emit(tc, io, with_bias)
    nc.compile()
    _prog_cache[key] = (nc, io)
    return nc, io


def kernel(x, gw, gb, w1, b1, w2, b2, _trace=False):
    x = np.ascontiguousarray(np.asarray(x, dtype=np.float32).reshape(NTOK,
                                                                     DIM))
    gw = np.asarray(gw, np.float32)
    gb = np.asarray(gb, np.float32)
    w1 = np.asarray(w1, np.float32)
    b1 = np.asarray(b1, np.float32)
    w2 = np.asarray(w2, np.float32)
    b2 = np.asarray(b2, np.float32)
    with_bias = bool(gb.any() or b1.any() or b2.any())
    nc, io = build(with_bias)
    in_maps = []
    for c in range(E):
        gwr = np.roll(gw, -c, axis=0)  # this core's expert at row 0
        gwp = np.zeros((16, DIM), np.float32)
        gwp[:E] = gwr
        m = {"x": x, "w1": np.ascontiguousarray(w1[c]),
             "w2": np.ascontiguousarray(w2[c]), "gwp": gwp}
        if with_bias:
            m["gb"] = np.ascontiguousarray(np.roll(gb, -c))
            m["b1"] = np.ascontiguousarray(b1[c])
            m["b2"] = np.ascontiguousarray(b2[c])
        in_maps.append(m)
    res = bass_utils.run_bass_kernel_spmd(nc, in_maps,
                                          core_ids=list(range(E)),
                                          trace=_trace)
    rows = NTOK // NRS // E
    full = np.zeros((NTOK, DIM), np.float32)
    for c in range(E):
        o = res.results[c]["out"]  # [NRS, rows, DIM]
        for j in range(NRS):
            r0 = j * (NTOK // NRS) + c * rows
            full[r0:r0 + rows] = o[j]
    out = full.reshape(2, 2048, DIM)
    if _trace:
        return out, res
    return out
